# revision 57
# baseline (speedup 1.0000x reference)
"""Banded (sparse) attention + projections on 8 Trainium2 NeuronCores.

Problem: nn_Attention_old_90211493085279
  x [2, 2048, 1024] -> qkv = x @ Wqkv, banded softmax(QK^T) V (half-width 8),
  out = attn @ Wproj + bproj.

Sharding: (batch x tokens) across 8 cores; each core owns 512 token rows plus
an 8-token halo of K/V context, so there are no collectives.

v2 pipeline (per core), tuned from the v1 trace:
  - inputs coalesced into a few large DMAs issued from different engines so
    the critical prefix (xh + wv) lands ~9us instead of ~21us
  - dummy warm-up matmuls run during the DMA wait so the PE HAM clock-gate is
    already at 8/8 when real work starts
  - fm loop software-pipelined: proj(k+2), av(k), scores(k+1) so the exp/mask
    latency of pair k hides under ~4us of independent PE work
  - softmax normalization is SBUF-only: cast stage tiles (denominator row
    included), reshape-DMA to [8,128], DVE reciprocal, collect-DMA, gpsimd
    partition_broadcast, one mul per head
  - output projection: m0's c0..c6 run right after av(6); remaining m-tiles
    interleave with the last normalization chain; c7 contributions last
  - bf16 output (halves the final DMA); host converts to f32
"""

import sys

sys.path.insert(0, "/opt/trn_rl_repo")

import ml_dtypes
import numpy as np

import concourse.bass as bass
import concourse.tile as tile
from concourse import bacc, mybir
from concourse.bass_utils import run_bass_kernel_spmd

F32 = mybir.dt.float32
BF16 = mybir.dt.bfloat16
AF = mybir.ActivationFunctionType

B, N, C, H, HD, W = 2, 2048, 1024, 16, 64, 8
SCALE = float(HD) ** -0.5
CORES = 8
TOK = 512            # token rows owned per core
HALO = TOK + 2 * W   # 528 k/v context tokens per core
NT = TOK // 128      # 4 row tiles of 128
WIN = 128 + 2 * W    # 144 k/v window per row tile
N_DUMMY = 33         # warm-up matmuls during the input DMA wait

_CACHE = {}


def _build_nc(dbg=False):
    nc = bacc.Bacc(None, target_bir_lowering=False)
    # all large inputs arrive pre-arranged as exact SBUF images [128, free]
    xhT = nc.dram_tensor("xhT", [128, 8 * HALO], BF16, kind="ExternalInput")
    wqk = nc.dram_tensor("wqk", [128, 16 * C], BF16, kind="ExternalInput")
    wv = nc.dram_tensor("wv", [128, 8 * C], BF16, kind="ExternalInput")
    wp = nc.dram_tensor("wp", [128, 8 * C], BF16, kind="ExternalInput")
    bp = nc.dram_tensor("bp", [128, 8], F32, kind="ExternalInput")
    mX = nc.dram_tensor("mX", [128, 2 * 1280], BF16, kind="ExternalInput")
    outT = nc.dram_tensor("outT", [C, TOK], BF16, kind="ExternalOutput")

    vsizes = [128, 128, 128, 128, 2 * W]

    with tile.TileContext(nc) as tc:
        with tc.tile_pool(name="persist", bufs=1) as pp, \
             tc.tile_pool(name="ps", bufs=1, space="PSUM") as ps, \
             tc.tile_pool(name="atp", bufs=2) as atp, \
             tc.tile_pool(name="stgp", bufs=2) as stgp, \
             tc.tile_pool(name="np_", bufs=2) as npp, \
             tc.tile_pool(name="outp", bufs=4) as outp:

            # ---- persistent SBUF arrays ----
            mask_x = pp.tile([128, 2 * 1280], BF16, tag="mask_x", name="mask_x")
            bias_sb = pp.tile([128, 8], F32, tag="bias", name="bias")
            xh = pp.tile([128, 8, HALO], BF16, tag="xh", name="xh")
            wv_sb = pp.tile([128, 2, 8, 512], BF16, tag="wv", name="wv")
            wqk_sb = pp.tile([128, 8, 8, 256], BF16, tag="wqk", name="wqk")
            wp_sb = pp.tile([128, 8, C], BF16, tag="wp", name="wp")
            qT = pp.tile([128, 8 * TOK], BF16, tag="qT", name="qT")
            kT = pp.tile([128, 8 * HALO], BF16, tag="kT", name="kT")
            v1 = [pp.tile([p, H, HD + 1], BF16, tag=f"v1_{t}", name=f"v1_{t}")
                  for t, p in enumerate(vsizes)]
            otn = [pp.tile([128, TOK], BF16, tag=f"otn{m}", name=f"otn{m}") for m in range(8)]
            zw = pp.tile([128, TOK], BF16, tag="zw", name="zw")
            dp = tc.alloc_tile_pool(name="dram", bufs=1, space="DRAM")

            # ---- contiguous input DMAs, ALL on the sync HWDGE queue in
            # strict priority order (the queues share the ~358GB/s per-core
            # DMA bandwidth, so a second queue would steal from the prefix).
            # wv image is packed [half | c | 512] so P2's first half-wave can
            # start after xh + wv-h0 only. ----
            # wqk image is fm-major [p, fm, c, 256] so early head pairs can
            # project before the whole weight set arrives
            nc.sync.dma_start(out=xh[:], in_=xhT[:])
            nc.sync.dma_start(out=wqk_sb[:, 0:2, :, :], in_=wqk[:, 0:2 * 2048])
            nc.sync.dma_start(out=wv_sb[:, 0, :, :], in_=wv[:, 0:8 * 512])
            nc.sync.dma_start(out=wv_sb[:, 1, :, :], in_=wv[:, 8 * 512:8 * 1024])
            nc.sync.dma_start(out=wqk_sb[:, 2:4, :, :], in_=wqk[:, 2 * 2048:4 * 2048])
            nc.sync.dma_start(out=mask_x[:], in_=mX[:])
            nc.scalar.dma_start(out=bias_sb[:], in_=bp[:])
            nc.sync.dma_start(out=wqk_sb[:, 4:8, :, :], in_=wqk[:, 4 * 2048:8 * 2048])
            nc.sync.dma_start(out=wp_sb[:], in_=wp[:])

            # ---- early memsets (no deps) + PE warm-up dummies ----
            nc.vector.memset(zw[:], 0.0)
            for t in range(5):
                nc.gpsimd.memset(v1[t][:, :, HD], 1.0)
            # atX buffers: per head e at 1280e, block t at 256t holds
            # [B(t-1) rows 0:16 | A(t)]; rows 16:128 of B sub-blocks must be
            # zero once (never overwritten), so the AV matmul can read a full
            # [128, 256] moving operand per block.
            atx = [atp.tile([128, 2 * 1280], BF16, tag="atX", name=f"atX{i}")
                   for i in range(2)]
            for i in range(2):
                nc.vector.memset(atx[i][:, :], 0.0)
            dmy = ps.tile([128, 2 * TOK], F32, tag="stA", name="stA_dummy")
            for i in range(N_DUMMY):
                nc.tensor.matmul(dmy[:, 0:512], zw[:, 0:128], zw[:],
                                 start=True, stop=True)

            # ---- P2: v projection (natural layout); ones col via memset ----
            def p2():
                for hh in range(2):
                    for t in range(5):
                        p = vsizes[t]
                        tagv = "otbE" if (5 * hh + t) % 2 == 0 else "otbO"
                        pv = ps.tile([128, 512], F32, tag=tagv, name=f"pv{hh}{t}")
                        for c in range(8):
                            nc.tensor.matmul(pv[:p, :], xh[:, c, 128 * t:128 * t + p],
                                             wv_sb[:, hh, c, :],
                                             start=(c == 0), stop=(c == 7))
                        nc.vector.tensor_copy(
                            v1[t][:, 8 * hh:8 * hh + 8, 0:HD],
                            pv[:p, :].rearrange("p (h d) -> p h d", d=HD))

            # ---- attention: software-pipelined over head pairs fm ----
            def proj(fm):
                # the last pair's copies go to vector so the scalar queue
                # reaches exp(7) (on the critical tail path) sooner
                eng = nc.vector if fm == 7 else nc.scalar
                cp = (nc.vector.tensor_copy if fm == 7 else
                      (lambda o, i: nc.scalar.copy(o, i)))
                pa = ps.tile([128, 512], F32, tag="pa", name=f"pa{fm}")
                for c in range(8):
                    nc.tensor.matmul(pa[:], wqk_sb[:, fm, c, 0:128],
                                     xh[:, c, W:W + TOK],
                                     start=(c == 0), stop=(c == 7))
                cp(qT[:, TOK * fm:TOK * (fm + 1)], pa[:])
                # pk/pb interleaved per c so each wqk chunk is loaded once
                pk = ps.tile([128, 512], F32, tag="pk", name=f"pk{fm}")
                pb = ps.tile([128, 512], F32, tag="pa", name=f"pb{fm}")
                for c in range(8):
                    nc.tensor.matmul(pk[:], wqk_sb[:, fm, c, 128:256],
                                     xh[:, c, 0:512],
                                     start=(c == 0), stop=(c == 7))
                    nc.tensor.matmul(pb[:, 0:2 * W],
                                     wqk_sb[:, fm, c, 128:256],
                                     xh[:, c, 512:528],
                                     start=(c == 0), stop=(c == 7))
                cp(kT[:, HALO * fm:HALO * fm + 512], pk[:])
                cp(kT[:, HALO * fm + 512:HALO * fm + 528], pb[:, 0:2 * W])

            sc_state = {}

            def scores(fm):
                stA = ps.tile([128, 2 * TOK], F32, tag="stA", name=f"stA{fm}")
                stBe = ps.tile([128, TOK], F32, tag="stBe", name=f"stBe{fm}")
                stBo = ps.tile([128, TOK], F32, tag="stBo", name=f"stBo{fm}")
                # interleave even/odd so the row-group pairs run concurrently
                for t in range(NT):
                    for e in range(2):
                        off = 64 * e
                        q_ap = qT[off:off + 64, TOK * fm + 128 * t:TOK * fm + 128 * t + 128]
                        k1 = kT[off:off + 64, HALO * fm + 128 * t:HALO * fm + 128 * t + 128]
                        nc.tensor.matmul(stA[:, TOK * e + 128 * t:TOK * e + 128 * (t + 1)],
                                         k1, q_ap, start=True, stop=True)
                for t in range(NT):
                    for e in range(2):
                        off = 64 * e
                        q_ap = qT[off:off + 64, TOK * fm + 128 * t:TOK * fm + 128 * t + 128]
                        k2 = kT[off:off + 64,
                                HALO * fm + 128 * t + 128:HALO * fm + 128 * t + WIN]
                        stB = stBe if e == 0 else stBo
                        nc.tensor.matmul(stB[0:2 * W, 128 * t:128 * (t + 1)], k2, q_ap,
                                         start=True, stop=True)
                ax = atx[fm % 2]
                axA = ax[:, :].rearrange("p (e t j) -> p e t j", e=2, j=256)
                axB = ax[0:2 * W, :].rearrange("p (e t j) -> p e t j", e=2, j=256)
                # A(t) into cols 256t+128; B(t) into block t+1 cols 0:128
                nc.scalar.activation(
                    axA[:, :, 0:4, 128:256],
                    stA[:].rearrange("p (e t j) -> p e t j", e=2, j=128), AF.Exp)
                nc.scalar.activation(
                    axB[:, 0, 1:5, 0:128],
                    stBe[0:2 * W, :].rearrange("p (t j) -> p t j", j=128), AF.Exp)
                nc.scalar.activation(
                    axB[:, 1, 1:5, 0:128],
                    stBo[0:2 * W, :].rearrange("p (t j) -> p t j", j=128), AF.Exp)
                # per-head halves so av(fm) e0 need not wait the full mul
                nc.vector.tensor_mul(ax[:, 0:1280], ax[:, 0:1280], mask_x[:, 0:1280])
                nc.vector.tensor_mul(ax[:, 1280:2560], ax[:, 1280:2560],
                                     mask_x[:, 1280:2560])
                sc_state[fm] = ax

            def av(fm):
                ax = sc_state.pop(fm)
                otbs = []
                for e in range(2):
                    h = 2 * fm + e
                    otb = ps.tile([HD + 1, TOK], F32,
                                  tag=("otbE" if e == 0 else "otbO"), name=f"otb{h}")
                    base = 1280 * e
                    # block 0: A(0) only; blocks 1..3: [B(t-1)|A(t)] fused
                    # N=256; block 4: B(3) only. Overlapping writes resolve
                    # via per-element has_written accumulation.
                    nc.tensor.matmul(otb[:, 0:128], v1[0][:, h, :],
                                     ax[:, base + 128:base + 256],
                                     start=True, stop=False, skip_group_check=True)
                    for t in range(1, 4):
                        nc.tensor.matmul(otb[:, 128 * (t - 1):128 * (t + 1)],
                                         v1[t][:, h, :],
                                         ax[:, base + 256 * t:base + 256 * (t + 1)],
                                         start=False, stop=False,
                                         skip_group_check=True)
                    nc.tensor.matmul(otb[:, 384:512], v1[4][0:2 * W, h, :],
                                     ax[0:2 * W, base + 1024:base + 1152],
                                     start=False, stop=True, skip_group_check=True)
                    otbs.append(otb)
                return norm(fm, otbs)

            def norm(fm, otbs):
                q = nc.sync
                # stage casts to bf16 (denominator row 64 included)
                stg = []
                for e in range(2):
                    s = stgp.tile([HD + 1, TOK], BF16, tag=("stE" if e == 0 else "stO"),
                                  name=f"stage{2 * fm + e}")
                    nc.vector.tensor_copy(s[:], otbs[e][:])
                    stg.append(s)
                # den8[4e + i, j] = stage_e[64, 128 i + j]
                den8 = npp.tile([8, 128], BF16, tag="den8", name=f"den8_{fm}")
                for e in range(2):
                    q.dma_start(
                        out=den8[4 * e:4 * e + 4, :],
                        in_=stg[e][64:65, :].rearrange("p (c j) -> p c j", j=128))
                rec8 = npp.tile([8, 128], BF16, tag="rec8", name=f"rec8_{fm}")
                with nc.allow_low_precision(reason="softmax denom in bf16; tol 2e-2"):
                    nc.vector.reciprocal(rec8[:], den8[:])
                # collect per-head reciprocal rows -> [1, 512] each
                recs, bcs = [], []
                for e in range(2):
                    r = npp.tile([1, TOK], BF16, tag=f"rec_{e}", name=f"rec{fm}_{e}")
                    q.dma_start(
                        out=r[:].rearrange("p (c j) -> p c j", j=128),
                        in_=rec8[4 * e:4 * e + 4, :])
                    recs.append(r)
                # broadcast each head's reciprocal over 64 partitions
                for e in range(2):
                    b = npp.tile([HD, TOK], BF16, tag=f"bc_{e}", name=f"bc{fm}_{e}")
                    nc.gpsimd.partition_broadcast(b[:, :], recs[e][0:1, :],
                                                  channels=HD)
                    bcs.append(b)
                def fin():
                    # emitted after the NEXT pair's mask-mul so these never
                    # block it on the vector queue (they wait on gpsimd)
                    nc.vector.tensor_mul(otn[fm][0:HD, :], stg[0][0:HD, :],
                                         bcs[0][:, :])
                    nc.vector.tensor_mul(otn[fm][HD:128, :], stg[1][0:HD, :],
                                         bcs[1][:, :])
                return fin

            p5_state = {}
            p5_stA = []

            def p5_open(m, tag, half=0):
                # accumulate c0..c5 only, so no open depends on otn6/otn7
                if tag == "stA":
                    # one shared 2-bank tile; m2 writes half 0, m6 half 1
                    if not p5_stA:
                        p5_stA.append(ps.tile([128, 2 * TOK], F32, tag=tag,
                                              name=f"pf{m}"))
                    pf = p5_stA[0][:, 512 * half:512 * (half + 1)]
                else:
                    pf = ps.tile([128, 512], F32, tag=tag, name=f"pf{m}")[:]
                for c in range(6):
                    nc.tensor.matmul(pf, wp_sb[:, c, 128 * m:128 * (m + 1)], otn[c][:],
                                     start=(c == 0), stop=False)
                p5_state[m] = pf

            def p5_c6(m):
                nc.tensor.matmul(p5_state[m], wp_sb[:, 6, 128 * m:128 * (m + 1)],
                                 otn[6][:], start=False, stop=False)

            def p5_close(m):
                pf = p5_state.pop(m)
                nc.tensor.matmul(pf, wp_sb[:, 7, 128 * m:128 * (m + 1)], otn[7][:],
                                 start=False, stop=True)
                ob = outp.tile([128, 512], BF16, tag="ob", name=f"ob{m}")
                if m % 2 == 0:
                    nc.vector.tensor_scalar_add(ob[:], pf, bias_sb[:, m:m + 1])
                    nc.sync.dma_start(out=outT[128 * m:128 * (m + 1), :], in_=ob[:])
                else:
                    nc.scalar.add(ob[:], pf, bias_sb[:, m:m + 1])
                    nc.scalar.dma_start(out=outT[128 * m:128 * (m + 1), :], in_=ob[:])

            # depth-2 pipeline; fin(k) trails by 2 slots so the otn muls
            # (which wait on gpsimd broadcasts) never head-of-line block the
            # next pair's mask-mul or stage casts on the vector queue.
            # proj(0)/proj(1) run before P2: they need only xh + the first
            # wqk slice, which land a few us before wv completes.
            fins = {}
            proj(0)
            proj(1)
            p2()
            scores(0)
            for k in range(6):
                proj(k + 2)
                fins[k] = av(k)
                scores(k + 1)
                if k - 1 in fins:
                    fins.pop(k - 1)()
            scores(7)
            fins[6] = av(6)
            fins.pop(5)()
            p5_open(0, "pa")
            p5_open(1, "pk")
            fins[7] = av(7)
            fins.pop(6)()
            fins.pop(7)()
            # P5: all eight m-tiles open (c0..c5) on freed banks, then the c6
            # sweep, then c7+bias+store — so only the last 16 matmuls depend
            # on the two final normalization chains
            p5_open(2, "stA", half=0)
            p5_open(3, "stBe")
            p5_open(4, "stBo")
            p5_open(5, "otbE")
            p5_open(6, "stA", half=1)
            p5_open(7, "otbO")
            for m in range(8):
                p5_c6(m)
            for m in range(8):
                p5_close(m)

    nc.finalize()
    return nc


def _get_nc(dbg=False):
    key = ("nc", dbg)
    if key not in _CACHE:
        _CACHE[key] = _build_nc(dbg)
    return _CACHE[key]


def _band_mask_np(n, w):
    i = np.arange(n)[:, None]
    j = np.arange(n)[None, :]
    lo = np.where(i <= w, 0, i - w)
    hi = np.where(n - i <= w, n - 1, i + w)
    return (j >= lo) & (j <= hi)


def _make_in_maps(x, Wqkv, Wproj, bproj):
    x = np.ascontiguousarray(np.asarray(x, dtype=np.float32))
    Wqkv = np.asarray(Wqkv, dtype=np.float32)
    Wproj = np.ascontiguousarray(np.asarray(Wproj, dtype=np.float32))
    bproj = np.asarray(bproj, dtype=np.float32)

    def sbuf_img(a):
        # [1024, X] -> SBUF image [128, 8*X]: img[p, X*c+j] = a[128c+p, j]
        xdim = a.shape[1]
        return np.ascontiguousarray(
            a.reshape(8, 128, xdim).transpose(1, 0, 2).reshape(128, 8 * xdim)
        ).astype(ml_dtypes.bfloat16)

    # fm-major packing: col block fm = [q_fm * SCALE | k_fm]
    wq = Wqkv[:, :C] * np.float32(SCALE)
    wk = Wqkv[:, C:2 * C]
    blocks = []
    for fm in range(8):
        blocks.append(wq[:, 128 * fm:128 * (fm + 1)])
        blocks.append(wk[:, 128 * fm:128 * (fm + 1)])
    # wqk image fm-major: [p, fm, c, 256]
    wqk_host = np.ascontiguousarray(
        np.concatenate(blocks, axis=1).reshape(8, 128, 8, 256)
        .transpose(1, 2, 0, 3).reshape(128, 16 * C)).astype(ml_dtypes.bfloat16)
    # wv image packed [p, half, c, 512] so the halves are contiguous
    wv_host = np.ascontiguousarray(
        Wqkv[:, 2 * C:].reshape(8, 128, 2, 512).transpose(1, 2, 0, 3)
        .reshape(128, 8192)).astype(ml_dtypes.bfloat16)
    wp_host = sbuf_img(Wproj)
    bp_host = np.ascontiguousarray(bproj.reshape(8, 128).T)
    band = _band_mask_np(N, W)

    in_maps = []
    for core in range(CORES):
        b, qt = divmod(core, NT)
        g0 = qt * TOK
        xhrows = np.zeros((HALO, C), np.float32)
        s = max(0, g0 - W)
        e = min(N, g0 + TOK + W)
        xhrows[s - (g0 - W):e - (g0 - W)] = x[b, s:e]
        xhT_host = sbuf_img(xhrows.T)

        mAh = np.zeros((128, TOK), np.float32)
        mBh = np.zeros((2 * W, TOK), np.float32)
        for t in range(NT):
            i = g0 + 128 * t + np.arange(128)[None, :]
            jw = (g0 - W) + 128 * t + np.arange(WIN)[:, None]
            valid = (jw >= 0) & (jw < N)
            mm = band[i, np.clip(jw, 0, N - 1)] & valid
            mAh[:, 128 * t:128 * (t + 1)] = mm[:128]
            mBh[:, 128 * t:128 * (t + 1)] = mm[128:]
        # mask in atX layout: block t = [mB(t-1) rows 0:16 | mA(t)]
        mx1 = np.zeros((128, 1280), np.float32)
        for t in range(NT):
            mx1[:, 256 * t + 128:256 * (t + 1)] = mAh[:, 128 * t:128 * (t + 1)]
            mx1[0:2 * W, 256 * (t + 1):256 * (t + 1) + 128] = \
                mBh[:, 128 * t:128 * (t + 1)]
        in_maps.append({
            "xhT": xhT_host, "wqk": wqk_host, "wv": wv_host,
            "wp": wp_host, "bp": bp_host,
            "mX": np.concatenate([mx1, mx1], axis=1).astype(ml_dtypes.bfloat16),
        })
    return in_maps


def run_spmd(x, Wqkv, Wproj, bproj, dbg=False, **kw):
    """Run the SPMD kernel; returns (output, BassKernelResults)."""
    nc = _get_nc(dbg)
    in_maps = _make_in_maps(x, Wqkv, Wproj, bproj)
    res = run_bass_kernel_spmd(nc, in_maps, list(range(CORES)), **kw)
    outT = np.concatenate(
        [np.asarray(res.results[i]["outT"], dtype=np.float32) for i in range(CORES)],
        axis=1)
    out = np.ascontiguousarray(outT.T).reshape(B, N, C)
    return out, res


def kernel(x, Wqkv, Wproj, bproj):
    out, _ = run_spmd(x, Wqkv, Wproj, bproj)
    return out


# revision 59
# speedup vs baseline: 1.0195x; 1.0195x over previous
"""Banded (sparse) attention + projections on 8 Trainium2 NeuronCores.

Problem: nn_Attention_old_90211493085279
  x [2, 2048, 1024] -> qkv = x @ Wqkv, banded softmax(QK^T) V (half-width 8),
  out = attn @ Wproj + bproj.

Sharding: (batch x tokens) across 8 cores; each core owns 512 token rows plus
an 8-token halo of K/V context, so there are no collectives.

v2 pipeline (per core), tuned from the v1 trace:
  - inputs coalesced into a few large DMAs issued from different engines so
    the critical prefix (xh + wv) lands ~9us instead of ~21us
  - dummy warm-up matmuls run during the DMA wait so the PE HAM clock-gate is
    already at 8/8 when real work starts
  - fm loop software-pipelined: proj(k+2), av(k), scores(k+1) so the exp/mask
    latency of pair k hides under ~4us of independent PE work
  - softmax normalization is SBUF-only: cast stage tiles (denominator row
    included), reshape-DMA to [8,128], DVE reciprocal, collect-DMA, gpsimd
    partition_broadcast, one mul per head
  - output projection: m0's c0..c6 run right after av(6); remaining m-tiles
    interleave with the last normalization chain; c7 contributions last
  - bf16 output (halves the final DMA); host converts to f32
"""

import sys

sys.path.insert(0, "/opt/trn_rl_repo")

import ml_dtypes
import numpy as np

import concourse.bass as bass
import concourse.tile as tile
from concourse import bacc, mybir
from concourse.bass_utils import run_bass_kernel_spmd

F32 = mybir.dt.float32
BF16 = mybir.dt.bfloat16
AF = mybir.ActivationFunctionType

B, N, C, H, HD, W = 2, 2048, 1024, 16, 64, 8
SCALE = float(HD) ** -0.5
CORES = 8
TOK = 512            # token rows owned per core
HALO = TOK + 2 * W   # 528 k/v context tokens per core
NT = TOK // 128      # 4 row tiles of 128
WIN = 128 + 2 * W    # 144 k/v window per row tile
N_DUMMY = 33         # warm-up matmuls during the input DMA wait

_CACHE = {}


def _build_nc(dbg=False):
    nc = bacc.Bacc(None, target_bir_lowering=False)
    # all large inputs arrive pre-arranged as exact SBUF images [128, free]
    xhT = nc.dram_tensor("xhT", [128, 8 * HALO], BF16, kind="ExternalInput")
    wqk = nc.dram_tensor("wqk", [128, 16 * C], BF16, kind="ExternalInput")
    wv = nc.dram_tensor("wv", [128, 8 * C], BF16, kind="ExternalInput")
    wp = nc.dram_tensor("wp", [128, 8 * C], BF16, kind="ExternalInput")
    bp = nc.dram_tensor("bp", [128, 8], F32, kind="ExternalInput")
    mX = nc.dram_tensor("mX", [128, 2 * 1280], BF16, kind="ExternalInput")
    outT = nc.dram_tensor("outT", [C, TOK], BF16, kind="ExternalOutput")

    vsizes = [128, 128, 128, 128, 2 * W]

    with tile.TileContext(nc) as tc:
        with tc.tile_pool(name="persist", bufs=1) as pp, \
             tc.tile_pool(name="ps", bufs=1, space="PSUM") as ps, \
             tc.tile_pool(name="atp", bufs=2) as atp, \
             tc.tile_pool(name="stgp", bufs=2) as stgp, \
             tc.tile_pool(name="np_", bufs=2) as npp, \
             tc.tile_pool(name="outp", bufs=4) as outp:

            # ---- persistent SBUF arrays ----
            mask_x = pp.tile([128, 2 * 1280], BF16, tag="mask_x", name="mask_x")
            bias_sb = pp.tile([128, 8], F32, tag="bias", name="bias")
            xh = pp.tile([128, 8, HALO], BF16, tag="xh", name="xh")
            wv_sb = pp.tile([128, 2, 8, 512], BF16, tag="wv", name="wv")
            wqk_sb = pp.tile([128, 8, 8, 256], BF16, tag="wqk", name="wqk")
            wp_sb = pp.tile([128, 8, C], BF16, tag="wp", name="wp")
            qT = pp.tile([128, 8 * TOK], BF16, tag="qT", name="qT")
            kT = pp.tile([128, 8 * HALO], BF16, tag="kT", name="kT")
            v1 = [pp.tile([p, H, HD + 1], BF16, tag=f"v1_{t}", name=f"v1_{t}")
                  for t, p in enumerate(vsizes)]
            otn = [pp.tile([128, TOK], BF16, tag=f"otn{m}", name=f"otn{m}") for m in range(8)]
            zw = pp.tile([128, TOK], BF16, tag="zw", name="zw")
            dp = tc.alloc_tile_pool(name="dram", bufs=1, space="DRAM")

            # ---- contiguous input DMAs, ALL on the sync HWDGE queue in
            # strict priority order (the queues share the ~358GB/s per-core
            # DMA bandwidth, so a second queue would steal from the prefix).
            # wv image is packed [half | c | 512] so P2's first half-wave can
            # start after xh + wv-h0 only. ----
            # wqk image is fm-major [p, fm, c, 256] so early head pairs can
            # project before the whole weight set arrives
            nc.sync.dma_start(out=xh[:], in_=xhT[:])
            nc.sync.dma_start(out=wqk_sb[:, 0:2, :, :], in_=wqk[:, 0:2 * 2048])
            nc.sync.dma_start(out=wv_sb[:, 0, :, :], in_=wv[:, 0:8 * 512])
            nc.sync.dma_start(out=wv_sb[:, 1, :, :], in_=wv[:, 8 * 512:8 * 1024])
            nc.sync.dma_start(out=wqk_sb[:, 2:4, :, :], in_=wqk[:, 2 * 2048:4 * 2048])
            nc.sync.dma_start(out=mask_x[:], in_=mX[:])
            nc.scalar.dma_start(out=bias_sb[:], in_=bp[:])
            nc.sync.dma_start(out=wqk_sb[:, 4:8, :, :], in_=wqk[:, 4 * 2048:8 * 2048])
            nc.sync.dma_start(out=wp_sb[:], in_=wp[:])

            # ---- early memsets (no deps) + PE warm-up dummies ----
            nc.vector.memset(zw[:], 0.0)
            for t in range(5):
                nc.gpsimd.memset(v1[t][:, :, HD], 1.0)
            # atX buffers: per head e at 1280e, block t at 256t holds
            # [B(t-1) rows 0:16 | A(t)]; rows 16:128 of B sub-blocks must be
            # zero once (never overwritten), so the AV matmul can read a full
            # [128, 256] moving operand per block.
            atx = [atp.tile([128, 2 * 1280], BF16, tag="atX", name=f"atX{i}")
                   for i in range(2)]
            for i in range(2):
                nc.vector.memset(atx[i][:, :], 0.0)
            dmy = ps.tile([128, 2 * TOK], F32, tag="stA", name="stA_dummy")
            for i in range(N_DUMMY):
                nc.tensor.matmul(dmy[:, 0:512], zw[:, 0:128], zw[:],
                                 start=True, stop=True)

            # ---- P2: v projection (natural layout); ones col via memset ----
            def p2():
                for hh in range(2):
                    for t in range(5):
                        p = vsizes[t]
                        tagv = "otbE" if (5 * hh + t) % 2 == 0 else "otbO"
                        pv = ps.tile([128, 512], F32, tag=tagv, name=f"pv{hh}{t}")
                        for c in range(8):
                            nc.tensor.matmul(pv[:p, :], xh[:, c, 128 * t:128 * t + p],
                                             wv_sb[:, hh, c, :],
                                             start=(c == 0), stop=(c == 7))
                        nc.vector.tensor_copy(
                            v1[t][:, 8 * hh:8 * hh + 8, 0:HD],
                            pv[:p, :].rearrange("p (h d) -> p h d", d=HD))

            # ---- attention: software-pipelined over head pairs fm ----
            def proj(fm):
                # the last pair's copies go to vector so the scalar queue
                # reaches exp(7) (on the critical tail path) sooner
                eng = nc.vector if fm == 7 else nc.scalar
                cp = (nc.vector.tensor_copy if fm == 7 else
                      (lambda o, i: nc.scalar.copy(o, i)))
                pa = ps.tile([128, 512], F32, tag="pa", name=f"pa{fm}")
                for c in range(8):
                    nc.tensor.matmul(pa[:], wqk_sb[:, fm, c, 0:128],
                                     xh[:, c, W:W + TOK],
                                     start=(c == 0), stop=(c == 7))
                cp(qT[:, TOK * fm:TOK * (fm + 1)], pa[:])
                # pk/pb interleaved per c so each wqk chunk is loaded once
                pk = ps.tile([128, 512], F32, tag="pk", name=f"pk{fm}")
                pb = ps.tile([128, 512], F32, tag="pa", name=f"pb{fm}")
                for c in range(8):
                    nc.tensor.matmul(pk[:], wqk_sb[:, fm, c, 128:256],
                                     xh[:, c, 0:512],
                                     start=(c == 0), stop=(c == 7))
                    nc.tensor.matmul(pb[:, 0:2 * W],
                                     wqk_sb[:, fm, c, 128:256],
                                     xh[:, c, 512:528],
                                     start=(c == 0), stop=(c == 7))
                cp(kT[:, HALO * fm:HALO * fm + 512], pk[:])
                cp(kT[:, HALO * fm + 512:HALO * fm + 528], pb[:, 0:2 * W])

            sc_state = {}

            def scores(fm):
                stA = ps.tile([128, 2 * TOK], F32, tag="stA", name=f"stA{fm}")
                stBe = ps.tile([128, TOK], F32, tag="stBe", name=f"stBe{fm}")
                stBo = ps.tile([128, TOK], F32, tag="stBo", name=f"stBo{fm}")
                # interleave even/odd so the row-group pairs run concurrently
                for t in range(NT):
                    for e in range(2):
                        off = 64 * e
                        q_ap = qT[off:off + 64, TOK * fm + 128 * t:TOK * fm + 128 * t + 128]
                        k1 = kT[off:off + 64, HALO * fm + 128 * t:HALO * fm + 128 * t + 128]
                        nc.tensor.matmul(stA[:, TOK * e + 128 * t:TOK * e + 128 * (t + 1)],
                                         k1, q_ap, start=True, stop=True)
                for t in range(NT):
                    for e in range(2):
                        off = 64 * e
                        q_ap = qT[off:off + 64, TOK * fm + 128 * t:TOK * fm + 128 * t + 128]
                        k2 = kT[off:off + 64,
                                HALO * fm + 128 * t + 128:HALO * fm + 128 * t + WIN]
                        stB = stBe if e == 0 else stBo
                        nc.tensor.matmul(stB[0:2 * W, 128 * t:128 * (t + 1)], k2, q_ap,
                                         start=True, stop=True)
                ax = atx[fm % 2]
                axA = ax[:, :].rearrange("p (e t j) -> p e t j", e=2, j=256)
                axB = ax[0:2 * W, :].rearrange("p (e t j) -> p e t j", e=2, j=256)
                # A(t) into cols 256t+128; B(t) into block t+1 cols 0:128
                nc.scalar.activation(
                    axA[:, :, 0:4, 128:256],
                    stA[:].rearrange("p (e t j) -> p e t j", e=2, j=128), AF.Exp)
                nc.scalar.activation(
                    axB[:, 0, 1:5, 0:128],
                    stBe[0:2 * W, :].rearrange("p (t j) -> p t j", j=128), AF.Exp)
                nc.scalar.activation(
                    axB[:, 1, 1:5, 0:128],
                    stBo[0:2 * W, :].rearrange("p (t j) -> p t j", j=128), AF.Exp)
                # per-head halves so av(fm) e0 need not wait the full mul
                nc.vector.tensor_mul(ax[:, 0:1280], ax[:, 0:1280], mask_x[:, 0:1280])
                nc.vector.tensor_mul(ax[:, 1280:2560], ax[:, 1280:2560],
                                     mask_x[:, 1280:2560])
                sc_state[fm] = ax

            def av(fm):
                ax = sc_state.pop(fm)
                otbs = []
                for e in range(2):
                    h = 2 * fm + e
                    otb = ps.tile([HD + 1, TOK], F32,
                                  tag=("otbE" if e == 0 else "otbO"), name=f"otb{h}")
                    base = 1280 * e
                    # block 0: A(0) only; blocks 1..3: [B(t-1)|A(t)] fused
                    # N=256; block 4: B(3) only. Overlapping writes resolve
                    # via per-element has_written accumulation.
                    nc.tensor.matmul(otb[:, 0:128], v1[0][:, h, :],
                                     ax[:, base + 128:base + 256],
                                     start=True, stop=False, skip_group_check=True)
                    for t in range(1, 4):
                        nc.tensor.matmul(otb[:, 128 * (t - 1):128 * (t + 1)],
                                         v1[t][:, h, :],
                                         ax[:, base + 256 * t:base + 256 * (t + 1)],
                                         start=False, stop=False,
                                         skip_group_check=True)
                    nc.tensor.matmul(otb[:, 384:512], v1[4][0:2 * W, h, :],
                                     ax[0:2 * W, base + 1024:base + 1152],
                                     start=False, stop=True, skip_group_check=True)
                    otbs.append(otb)
                return norm(fm, otbs)

            def norm(fm, otbs):
                q = nc.sync
                # stage casts to bf16 (denominator row 64 included)
                stg = []
                for e in range(2):
                    s = stgp.tile([HD + 1, TOK], BF16, tag=("stE" if e == 0 else "stO"),
                                  name=f"stage{2 * fm + e}")
                    nc.vector.tensor_copy(s[:], otbs[e][:])
                    stg.append(s)
                # den8[4e + i, j] = stage_e[64, 128 i + j]
                den8 = npp.tile([8, 128], BF16, tag="den8", name=f"den8_{fm}")
                for e in range(2):
                    q.dma_start(
                        out=den8[4 * e:4 * e + 4, :],
                        in_=stg[e][64:65, :].rearrange("p (c j) -> p c j", j=128))
                rec8 = npp.tile([8, 128], BF16, tag="rec8", name=f"rec8_{fm}")
                with nc.allow_low_precision(reason="softmax denom in bf16; tol 2e-2"):
                    nc.vector.reciprocal(rec8[:], den8[:])
                # collect per-head reciprocal rows -> [1, 512] each
                recs, bcs = [], []
                for e in range(2):
                    r = npp.tile([1, TOK], BF16, tag=f"rec_{e}", name=f"rec{fm}_{e}")
                    q.dma_start(
                        out=r[:].rearrange("p (c j) -> p c j", j=128),
                        in_=rec8[4 * e:4 * e + 4, :])
                    recs.append(r)
                # broadcast each head's reciprocal over 64 partitions
                for e in range(2):
                    b = npp.tile([HD, TOK], BF16, tag=f"bc_{e}", name=f"bc{fm}_{e}")
                    nc.gpsimd.partition_broadcast(b[:, :], recs[e][0:1, :],
                                                  channels=HD)
                    bcs.append(b)
                def fin():
                    # emitted after the NEXT pair's mask-mul so these never
                    # block it on the vector queue (they wait on gpsimd)
                    nc.vector.tensor_mul(otn[fm][0:HD, :], stg[0][0:HD, :],
                                         bcs[0][:, :])
                    nc.vector.tensor_mul(otn[fm][HD:128, :], stg[1][0:HD, :],
                                         bcs[1][:, :])
                return fin

            p5_state = {}
            p5_stA = []

            def p5_open(m, tag, half=0):
                # accumulate c0..c4 only, so no open depends on otn5/6/7
                if tag == "stA":
                    # one shared 2-bank tile; m2 writes half 0, m6 half 1
                    if not p5_stA:
                        p5_stA.append(ps.tile([128, 2 * TOK], F32, tag=tag,
                                              name=f"pf{m}"))
                    pf = p5_stA[0][:, 512 * half:512 * (half + 1)]
                else:
                    pf = ps.tile([128, 512], F32, tag=tag, name=f"pf{m}")[:]
                for c in range(5):
                    nc.tensor.matmul(pf, wp_sb[:, c, 128 * m:128 * (m + 1)], otn[c][:],
                                     start=(c == 0), stop=False)
                p5_state[m] = pf

            def p5_c(m, c):
                nc.tensor.matmul(p5_state[m], wp_sb[:, c, 128 * m:128 * (m + 1)],
                                 otn[c][:], start=False, stop=False)

            def p5_close(m):
                pf = p5_state.pop(m)
                nc.tensor.matmul(pf, wp_sb[:, 7, 128 * m:128 * (m + 1)], otn[7][:],
                                 start=False, stop=True)
                ob = outp.tile([128, 512], BF16, tag="ob", name=f"ob{m}")
                if m % 2 == 0:
                    nc.vector.tensor_scalar_add(ob[:], pf, bias_sb[:, m:m + 1])
                    nc.sync.dma_start(out=outT[128 * m:128 * (m + 1), :], in_=ob[:])
                else:
                    nc.scalar.add(ob[:], pf, bias_sb[:, m:m + 1])
                    nc.scalar.dma_start(out=outT[128 * m:128 * (m + 1), :], in_=ob[:])

            # depth-2 pipeline; fin(k) trails by 2 slots so the otn muls
            # (which wait on gpsimd broadcasts) never head-of-line block the
            # next pair's mask-mul or stage casts on the vector queue.
            # proj(0)/proj(1) run before P2: they need only xh + the first
            # wqk slice, which land a few us before wv completes.
            fins = {}
            proj(0)
            proj(1)
            p2()
            scores(0)
            for k in range(6):
                proj(k + 2)
                fins[k] = av(k)
                scores(k + 1)
                if k - 1 in fins:
                    fins.pop(k - 1)()
            scores(7)
            fins[6] = av(6)
            fins.pop(5)()
            p5_open(0, "pa")
            p5_open(1, "pk")
            fins[7] = av(7)
            fins.pop(6)()
            fins.pop(7)()
            # P5: all eight m-tiles open (c0..c5) on freed banks, then the c6
            # sweep, then c7+bias+store — so only the last 16 matmuls depend
            # on the two final normalization chains
            p5_open(2, "stA", half=0)
            p5_open(3, "stBe")
            p5_open(4, "stBo")
            p5_open(5, "otbE")
            p5_open(6, "stA", half=1)
            p5_open(7, "otbO")
            for c in (5, 6):
                for m in range(8):
                    p5_c(m, c)
            for m in range(8):
                p5_close(m)

    nc.finalize()
    return nc


def _get_nc(dbg=False):
    key = ("nc", dbg)
    if key not in _CACHE:
        _CACHE[key] = _build_nc(dbg)
    return _CACHE[key]


def _band_mask_np(n, w):
    i = np.arange(n)[:, None]
    j = np.arange(n)[None, :]
    lo = np.where(i <= w, 0, i - w)
    hi = np.where(n - i <= w, n - 1, i + w)
    return (j >= lo) & (j <= hi)


def _make_in_maps(x, Wqkv, Wproj, bproj):
    x = np.ascontiguousarray(np.asarray(x, dtype=np.float32))
    Wqkv = np.asarray(Wqkv, dtype=np.float32)
    Wproj = np.ascontiguousarray(np.asarray(Wproj, dtype=np.float32))
    bproj = np.asarray(bproj, dtype=np.float32)

    def sbuf_img(a):
        # [1024, X] -> SBUF image [128, 8*X]: img[p, X*c+j] = a[128c+p, j]
        xdim = a.shape[1]
        return np.ascontiguousarray(
            a.reshape(8, 128, xdim).transpose(1, 0, 2).reshape(128, 8 * xdim)
        ).astype(ml_dtypes.bfloat16)

    # fm-major packing: col block fm = [q_fm * SCALE | k_fm]
    wq = Wqkv[:, :C] * np.float32(SCALE)
    wk = Wqkv[:, C:2 * C]
    blocks = []
    for fm in range(8):
        blocks.append(wq[:, 128 * fm:128 * (fm + 1)])
        blocks.append(wk[:, 128 * fm:128 * (fm + 1)])
    # wqk image fm-major: [p, fm, c, 256]
    wqk_host = np.ascontiguousarray(
        np.concatenate(blocks, axis=1).reshape(8, 128, 8, 256)
        .transpose(1, 2, 0, 3).reshape(128, 16 * C)).astype(ml_dtypes.bfloat16)
    # wv image packed [p, half, c, 512] so the halves are contiguous
    wv_host = np.ascontiguousarray(
        Wqkv[:, 2 * C:].reshape(8, 128, 2, 512).transpose(1, 2, 0, 3)
        .reshape(128, 8192)).astype(ml_dtypes.bfloat16)
    wp_host = sbuf_img(Wproj)
    bp_host = np.ascontiguousarray(bproj.reshape(8, 128).T)
    band = _band_mask_np(N, W)

    in_maps = []
    for core in range(CORES):
        b, qt = divmod(core, NT)
        g0 = qt * TOK
        xhrows = np.zeros((HALO, C), np.float32)
        s = max(0, g0 - W)
        e = min(N, g0 + TOK + W)
        xhrows[s - (g0 - W):e - (g0 - W)] = x[b, s:e]
        xhT_host = sbuf_img(xhrows.T)

        mAh = np.zeros((128, TOK), np.float32)
        mBh = np.zeros((2 * W, TOK), np.float32)
        for t in range(NT):
            i = g0 + 128 * t + np.arange(128)[None, :]
            jw = (g0 - W) + 128 * t + np.arange(WIN)[:, None]
            valid = (jw >= 0) & (jw < N)
            mm = band[i, np.clip(jw, 0, N - 1)] & valid
            mAh[:, 128 * t:128 * (t + 1)] = mm[:128]
            mBh[:, 128 * t:128 * (t + 1)] = mm[128:]
        # mask in atX layout: block t = [mB(t-1) rows 0:16 | mA(t)]
        mx1 = np.zeros((128, 1280), np.float32)
        for t in range(NT):
            mx1[:, 256 * t + 128:256 * (t + 1)] = mAh[:, 128 * t:128 * (t + 1)]
            mx1[0:2 * W, 256 * (t + 1):256 * (t + 1) + 128] = \
                mBh[:, 128 * t:128 * (t + 1)]
        in_maps.append({
            "xhT": xhT_host, "wqk": wqk_host, "wv": wv_host,
            "wp": wp_host, "bp": bp_host,
            "mX": np.concatenate([mx1, mx1], axis=1).astype(ml_dtypes.bfloat16),
        })
    return in_maps


def run_spmd(x, Wqkv, Wproj, bproj, dbg=False, **kw):
    """Run the SPMD kernel; returns (output, BassKernelResults)."""
    nc = _get_nc(dbg)
    in_maps = _make_in_maps(x, Wqkv, Wproj, bproj)
    res = run_bass_kernel_spmd(nc, in_maps, list(range(CORES)), **kw)
    outT = np.concatenate(
        [np.asarray(res.results[i]["outT"], dtype=np.float32) for i in range(CORES)],
        axis=1)
    out = np.ascontiguousarray(outT.T).reshape(B, N, C)
    return out, res


def kernel(x, Wqkv, Wproj, bproj):
    out, _ = run_spmd(x, Wqkv, Wproj, bproj)
    return out
